# revision 10
# baseline (speedup 1.0000x reference)
"""Trainium2 Bass kernel for a GCN layer:

    out = relu(segment_sum(feature[src], dst, N) @ W.T + b)

Strategy (8 NeuronCores, SPMD, no collectives):
  - Destination nodes are sharded across the 8 cores (12544 rows/core in
    tiles of 128). Each core owns all edges whose dst falls in its range.
  - Host prep buckets each core's edges by (dst tile, src window) and pads
    each bucket to a whole number of 128-edge chunks, giving every core an
    identical static schedule (single SPMD NEFF).
  - On device, each group of 5 dst tiles gathers its source rows from HBM
    with one dma_gather per src window (features stored bf16, 256B rows).
  - Per dst tile, a one-hot matmul segment-sums each 128-edge chunk
    directly in transposed orientation:
        aggT[f, d] += X_chunk[128e, f]^T @ S_chunk[128e, d]   (PSUM fp32)
    where S is built on the fly by comparing dst-local ids against an iota.
    This orientation makes the W matmul consume aggT with no transpose:
        out_tile[o, d] = relu(W[o,f] @ aggT[f, d] + b[o])     (bf16 PE)
  - Output is produced transposed per core ([128, 12544]) and re-assembled
    on the host. End-to-end rel err vs fp32 reference ~2.9e-3.
"""

import math

import ml_dtypes
import numpy as np

import concourse.bass as bass
import concourse.mybir as mybir
import concourse.tile as tile
from concourse import library_config
from concourse.bass_utils import run_bass_kernel_spmd

P = 128
F = 128
NCORES = 8
NWIN = 4  # src windows (dma_gather indices are int16, so <=32768 rows each)
G_TILES = 5  # dst tiles processed per group (bounds SBUF working set)
XBUFS = 3  # gather tile double/triple buffering depth

LAST_NC = None  # hooks for test.py's timing harness
LAST_IN_MAPS = None
LAST_CFG = None

SYNC_BUDGET = 1  # this walrus build rejects extra sync commands per inst


def _split_excess_waits(nc, budget=SYNC_BUDGET):
    """Walrus codegen here rejects instructions carrying more than `budget`
    total sync commands (sem waits + updates). Hoist excess waits onto NOPs
    inserted just before the instruction on the same engine (sequencers
    execute in order, so this is semantically identical)."""
    nsplit = 0
    for fn in nc.m.functions:
        for bb in fn.blocks:
            out = []
            for inst in bb.instructions:
                si = inst.sync_info
                if si is None or not si.on_wait:
                    out.append(inst)
                    continue
                allowed = max(0, budget - len(si.on_update))
                if len(si.on_wait) > allowed:
                    waits = list(si.on_wait)
                    excess = waits[allowed:]
                    del si.on_wait[allowed:]
                    for i in range(0, len(excess), budget):
                        n = mybir.InstNoOp(
                            name=f"{inst.name}-waitsplit-{i}", ins=[], outs=[])
                        n.engine = inst.engine
                        n.sync_info = mybir.SyncInfo(
                            on_wait=list(excess[i:i + budget]), on_update=[])
                        out.append(n)
                        nsplit += 1
                out.append(inst)
            bb.instructions[:] = out
    return nsplit


def _prep(feature, src, dst):
    """Bucket edges per (core, dst tile, src window); build per-core gather
    index / dst-local tensors with a schedule shared by all cores."""
    N = feature.shape[0]
    E = src.shape[0]
    T = math.ceil(N / (NCORES * P))  # dst tiles per core
    D = T * P  # dst rows per core
    WS = math.ceil(N / NWIN)  # src window rows
    assert WS <= 32768, f"window {WS} exceeds int16 gather index range"

    src = np.asarray(src, np.int64)
    dst = np.asarray(dst, np.int64)

    core_of = dst // D
    tile_of = (dst % D) // P
    dloc = dst % P  # D % P == 0, so dst-local-in-tile == dst % P
    win_of = src // WS
    widx = (src % WS).astype(np.int16)

    nkeys = NCORES * T * NWIN
    key = (core_of * T + tile_of) * NWIN + win_of
    counts = np.bincount(key, minlength=nkeys).reshape(NCORES, T, NWIN)

    # chunks per window, shared by every (core, tile): the static schedule
    K_w = np.maximum(1, -(-counts.max(axis=(0, 1)) // P)).astype(np.int64)
    CK = int(K_w.sum())
    woff = np.concatenate([[0], np.cumsum(K_w)[:-1]]).astype(np.int64)

    groups = [G_TILES] * (T // G_TILES)
    if T % G_TILES:
        groups.append(T % G_TILES)
    gstart = np.concatenate([[0], np.cumsum(groups)[:-1]]).astype(np.int64)

    # idx tensor column base per (group, window); cols are int16 columns of a
    # [128, TOTCOL] tensor, 16 indices per column (wrapped-16 layout)
    colbase = np.zeros((len(groups), NWIN), np.int64)
    acc = 0
    for g, Gg in enumerate(groups):
        for w in range(NWIN):
            colbase[g, w] = acc
            acc += Gg * int(K_w[w]) * (P // 16)
    TOTCOL = acc

    # rank of each edge within its (core,tile,window) bucket
    # (src-sorted bucket order was tried and is timing-neutral: the gather
    # is SDMA descriptor-rate bound, not HBM-locality bound)
    order = np.argsort(key, kind="stable")
    starts = np.concatenate([[0], np.cumsum(counts.reshape(-1))])[:-1]
    rank = np.arange(E, dtype=np.int64) - starts[key[order]]

    c_s = core_of[order]
    t_s = tile_of[order]
    w_s = win_of[order]
    k_s = rank // P
    p_s = rank % P

    # gather index tensor (int16, wrapped in 16 rows, replicated x8 later)
    idx16 = np.zeros((NCORES, 16, TOTCOL), np.int16)
    g_s = t_s // G_TILES
    tl_s = t_s % G_TILES
    j = (tl_s * K_w[w_s] + k_s) * P + p_s
    col = colbase[g_s, w_s] + j // 16
    idx16[c_s, j % 16, col] = widx[order]
    idx_full = np.ascontiguousarray(np.tile(idx16, (1, 8, 1)))  # [NCORES,128,TOTCOL]

    # dst-local ids per chunk slot ([-1] = padding -> zero one-hot row)
    dstl = np.full((NCORES, P, T * CK), -1.0, np.float32)
    dstl[c_s, p_s, t_s * CK + woff[w_s] + k_s] = dloc[order].astype(np.float32)
    dstl_bf = dstl.astype(ml_dtypes.bfloat16)

    # feature as bf16, padded to NWIN*WS rows
    fh = np.zeros((NWIN * WS, F), ml_dtypes.bfloat16)
    fh[:N] = np.asarray(feature, np.float32).astype(ml_dtypes.bfloat16)

    cfg = dict(
        N=N, T=T, D=D, WS=WS, K_w=[int(x) for x in K_w], CK=CK,
        woff=[int(x) for x in woff], groups=groups,
        gstart=[int(x) for x in gstart], colbase=colbase, TOTCOL=TOTCOL,
    )
    return cfg, fh, idx_full, dstl_bf


def _build(cfg, trivial=False, reps=1, gather_only=False, bulk=False):
    T, CK, TOTCOL, WS = cfg["T"], cfg["CK"], cfg["TOTCOL"], cfg["WS"]
    K_w, woff, groups, gstart = cfg["K_w"], cfg["woff"], cfg["groups"], cfg["gstart"]
    colbase = cfg["colbase"]
    bf16, f32, i16 = mybir.dt.bfloat16, mybir.dt.float32, mybir.dt.int16

    nc = bass.Bass("TRN2", target_bir_lowering=False, debug=False,
                   num_devices=NCORES, num_swdge_queues=NWIN)
    fh_d = nc.dram_tensor("fh", [NWIN * WS, F], bf16, kind="ExternalInput")
    idx_d = nc.dram_tensor("idx", [P, TOTCOL], i16, kind="ExternalInput")
    dstl_d = nc.dram_tensor("dstl", [P, T * CK], bf16, kind="ExternalInput")
    wt_d = nc.dram_tensor("wt", [F, F], bf16, kind="ExternalInput")  # W.T bf16
    b_d = nc.dram_tensor("bias", [F, 1], f32, kind="ExternalInput")
    iota_d = nc.dram_tensor("iota", [P, P], bf16, kind="ExternalInput")
    out_d = nc.dram_tensor("out", [P, T * P], f32, kind="ExternalOutput")

    if trivial:
        # matched-I/O no-op NEFF for test.py's dispatch-floor probe
        with tile.TileContext(nc) as tc:
            with tc.tile_pool(name="tp", bufs=1) as tp:
                t0 = tp.tile([F, F], bf16)
                nc.sync.dma_start(t0[:], wt_d.ap())
                t1 = tp.tile([F, F], f32)
                nc.vector.tensor_copy(t1[:], t0[:])
                nc.sync.dma_start(out_d.ap()[:, 0:F], t1[:])
        return nc

    # dma_gather (InstDMAGatherAnt) lives in the 'mlp' Q7 library; load it
    # before the Tile-scheduled region (same-engine program order holds).
    # This walrus build's visitInstISA needs the pseudo's 64-byte encoding
    # filled in, which plain load_library leaves empty.
    import concourse.bass_isa as bass_isa
    lib_inst = nc.gpsimd.load_library(library_config.mlp)
    _isa = nc.isa
    _po = _isa.get_enum("NEURON_ISA_TPB_PSEUDO_OPCODE")
    _bytes, _fix = bass_isa.isa_struct(
        _isa, _isa.Opcode.NEURON_ISA_TPB_OPCODE_PSEUDO_INST,
        {"pseudo_opcode":
         _po.NEURON_ISA_TPB_PSEUDO_OPCODE_PSEUDO_LIBRARY_RELOAD_INDEX.value,
         "lib_index": library_config.mlp.index})
    assert not _fix
    lib_inst.ins.instr = _bytes

    # One Pool register per distinct gather size (fresh to_reg per call
    # exhausts the register file at 80 calls).
    nidx_regs = {}
    for Gg in set(groups):
        for w in range(NWIN):
            v = Gg * K_w[w] * P
            if v not in nidx_regs:
                r = nc.gpsimd.alloc_register(f"nidx_{v}")
                nc.gpsimd.reg_mov(r, v)
                nidx_regs[v] = r

    with tile.TileContext(nc) as tc:
        with (
            tc.tile_pool(name="const", bufs=1) as cpool,
            tc.tile_pool(name="xp", bufs=XBUFS) as xpool,
            tc.tile_pool(name="work", bufs=2) as wpool,
            tc.tile_pool(name="ps", bufs=2, space="PSUM") as ppool,
        ):
            idx_sb = cpool.tile([P, TOTCOL], i16)
            nc.sync.dma_start(idx_sb[:], idx_d.ap())
            dstl_sb = cpool.tile([P, T * CK], bf16)
            nc.sync.dma_start(dstl_sb[:], dstl_d.ap())
            wt_sb = cpool.tile([F, F], bf16)
            nc.sync.dma_start(wt_sb[:], wt_d.ap())
            b_sb = cpool.tile([F, 1], f32)
            nc.sync.dma_start(b_sb[:], b_d.ap())
            iota_sb = cpool.tile([P, P], bf16)
            nc.sync.dma_start(iota_sb[:], iota_d.ap())

            for rep in range(reps):
                for g, Gg in enumerate(groups):
                    gs = gstart[g]
                    xw = []
                    for w in range(NWIN):
                        x = xpool.tile([P, Gg * K_w[w], F], bf16, tag=f"x{w}",
                                       name=f"x{w}_{g}_r{rep}")
                        nidx = Gg * K_w[w] * P
                        c0 = int(colbase[g, w])
                        if bulk:
                            # timing variant: same volume, ideal DMA pattern
                            nc.sync.dma_start(
                                x[:],
                                fh_d.ap()[w * WS:w * WS + nidx, :]
                                .rearrange("(p k) f -> p k f", p=P),
                            )
                        else:
                            nc.gpsimd.dma_gather(
                                out_ap=x[:],
                                in_ap=fh_d.ap()[w * WS:(w + 1) * WS, :],
                                idxs_ap=idx_sb[:, c0:c0 + nidx // 16],
                                num_idxs=nidx,
                                num_idxs_reg=nidx_regs[nidx],
                                elem_size=F,
                                single_packet=False,  # True faults for >~2K idx
                                queue_num=w,  # spread transfers over queues
                            )
                        xw.append(x)

                    ot = wpool.tile([P, Gg * P], f32, tag="ot",
                                    name=f"ot_{g}_r{rep}")
                    for tl in range(Gg):
                        t = gs + tl
                        if gather_only:
                            nc.vector.tensor_copy(
                                ot[:, tl * P:(tl + 1) * P], xw[0][:, tl, :])
                            continue
                        s_t = wpool.tile([P, CK * P], bf16, tag="s",
                                         name=f"s_{t}_r{rep}")
                        nc.vector.tensor_tensor(
                            out=s_t[:].rearrange("p (c f) -> p c f", f=P),
                            in0=dstl_sb[:, t * CK:(t + 1) * CK]
                            .rearrange("p (c o) -> p c o", o=1)
                            .to_broadcast([P, CK, P]),
                            in1=iota_sb[:]
                            .rearrange("p (o f) -> p o f", o=1)
                            .to_broadcast([P, CK, P]),
                            op=mybir.AluOpType.is_equal,
                        )
                        # aggT[f, d] += X_chunk[e, f]^T @ S_chunk[e, d]
                        aggT_ps = ppool.tile([F, P], f32, tag="aggT",
                                             name=f"aggT_{t}_r{rep}")
                        for w in range(NWIN):
                            for k in range(K_w[w]):
                                q = woff[w] + k
                                nc.tensor.matmul(
                                    aggT_ps[:],
                                    lhsT=xw[w][:, tl * K_w[w] + k, :],
                                    rhs=s_t[:, q * P:(q + 1) * P],
                                    start=(q == 0),
                                    stop=(q == CK - 1),
                                )
                        aggT_bf = wpool.tile([F, P], bf16, tag="aggTb",
                                             name=f"aggTb_{t}_r{rep}")
                        nc.vector.tensor_copy(aggT_bf[:], aggT_ps[:])
                        ot_p = ppool.tile([P, F], f32, tag="otp",
                                          name=f"otp_{t}_r{rep}")
                        nc.tensor.matmul(ot_p[:], lhsT=wt_sb[:], rhs=aggT_bf[:],
                                         start=True, stop=True)
                        nc.scalar.activation(
                            ot[:, tl * P:(tl + 1) * P], ot_p[:],
                            mybir.ActivationFunctionType.Relu,
                            bias=b_sb[:], scale=1.0,
                        )
                    nc.sync.dma_start(out_d.ap()[:, gs * P:(gs + Gg) * P], ot[:])
    return nc


def _in_maps(fh, idx_full, dstl_bf, W, b):
    wt = np.ascontiguousarray(np.asarray(W, np.float32).T
                              .astype(ml_dtypes.bfloat16))
    bcol = np.ascontiguousarray(np.asarray(b, np.float32).reshape(F, 1))
    iota = np.ascontiguousarray(
        np.tile(np.arange(P, dtype=np.float32)[None, :], (P, 1))
    ).astype(ml_dtypes.bfloat16)
    return [{
        "fh": fh,
        "idx": np.ascontiguousarray(idx_full[c]),
        "dstl": np.ascontiguousarray(dstl_bf[c]),
        "wt": wt,
        "bias": bcol,
        "iota": iota,
    } for c in range(NCORES)]


def kernel(feature, src, dst, W, b):
    global LAST_NC, LAST_IN_MAPS, LAST_CFG
    feature = np.asarray(feature)
    src = np.asarray(src)
    dst = np.asarray(dst)

    cfg, fh, idx_full, dstl_bf = _prep(feature, src, dst)
    in_maps = _in_maps(fh, idx_full, dstl_bf, W, b)

    # the walrus build in this container sporadically fails codegen; a fresh
    # build + recompile usually succeeds, so retry once before giving up
    results = None
    last_exc = None
    for attempt in range(2):
        try:
            nc = _build(cfg)
            _split_excess_waits(nc)
            LAST_NC, LAST_IN_MAPS, LAST_CFG = nc, in_maps, cfg
            res = run_bass_kernel_spmd(nc, in_maps, core_ids=list(range(NCORES)))
            results = res.results
            break
        except Exception as e:
            last_exc = e
    if results is None:
        raise last_exc

    D = cfg["D"]
    N = cfg["N"]
    out = np.empty((NCORES * D, F), np.float32)
    for c in range(NCORES):
        out[c * D:(c + 1) * D] = results[c]["out"].T
    return np.ascontiguousarray(out[:N])


# revision 14
# speedup vs baseline: 1.1190x; 1.1190x over previous
"""Trainium2 Bass kernel for a GCN layer:

    out = relu(segment_sum(feature[src], dst, N) @ W.T + b)

Strategy (8 NeuronCores, SPMD, no collectives):
  - Destination nodes are sharded across the 8 cores (12544 rows/core in
    tiles of 128). Each core owns all edges whose dst falls in its range.
  - Host prep buckets each core's edges by (dst tile, src window) and pads
    each bucket to a whole number of 128-edge chunks, giving every core an
    identical static schedule (single SPMD NEFF).
  - On device, each group of 5 dst tiles gathers its source rows from HBM
    with one dma_gather per src window (features stored bf16, 256B rows).
  - Per dst tile, a one-hot matmul segment-sums each 128-edge chunk
    directly in transposed orientation:
        aggT[f, d] += X_chunk[128e, f]^T @ S_chunk[128e, d]   (PSUM fp32)
    where S is built on the fly by comparing dst-local ids against an iota.
    This orientation makes the W matmul consume aggT with no transpose:
        out_tile[o, d] = relu(W[o,f] @ aggT[f, d] + b[o])     (bf16 PE)
  - Output is produced transposed per core ([128, 12544]) and re-assembled
    on the host. End-to-end rel err vs fp32 reference ~2.9e-3.
"""

import math

import ml_dtypes
import numpy as np

import concourse.bass as bass
import concourse.mybir as mybir
import concourse.tile as tile
from concourse import library_config
from concourse.bass_utils import run_bass_kernel_spmd

P = 128
F = 128
NCORES = 8
NWIN = 4  # src windows (dma_gather indices are int16, so <=32768 rows each)
G_TILES = 5  # dst tiles processed per group (bounds SBUF working set)
XBUFS = 3  # gather tile double/triple buffering depth

LAST_NC = None  # hooks for test.py's timing harness
LAST_IN_MAPS = None
LAST_CFG = None

SYNC_BUDGET = 1  # this walrus build rejects extra sync commands per inst


def _split_excess_waits(nc, budget=SYNC_BUDGET):
    """Walrus codegen here rejects instructions carrying more than `budget`
    total sync commands (sem waits + updates). Hoist excess waits onto NOPs
    inserted just before the instruction on the same engine (sequencers
    execute in order, so this is semantically identical)."""
    nsplit = 0
    for fn in nc.m.functions:
        for bb in fn.blocks:
            out = []
            for inst in bb.instructions:
                si = inst.sync_info
                if si is None or not si.on_wait:
                    out.append(inst)
                    continue
                allowed = max(0, budget - len(si.on_update))
                if len(si.on_wait) > allowed:
                    waits = list(si.on_wait)
                    excess = waits[allowed:]
                    del si.on_wait[allowed:]
                    for i in range(0, len(excess), budget):
                        n = mybir.InstNoOp(
                            name=f"{inst.name}-waitsplit-{i}", ins=[], outs=[])
                        n.engine = inst.engine
                        n.sync_info = mybir.SyncInfo(
                            on_wait=list(excess[i:i + budget]), on_update=[])
                        out.append(n)
                        nsplit += 1
                out.append(inst)
            bb.instructions[:] = out
    return nsplit


def _prep(feature, src, dst):
    """Bucket edges per (core, dst tile, src window); build per-core gather
    index / dst-local tensors with a schedule shared by all cores."""
    N = feature.shape[0]
    E = src.shape[0]
    T = math.ceil(N / (NCORES * P))  # dst tiles per core
    D = T * P  # dst rows per core
    WS = math.ceil(N / NWIN)  # src window rows
    assert WS <= 32768, f"window {WS} exceeds int16 gather index range"

    src = np.asarray(src, np.int64)
    dst = np.asarray(dst, np.int64)

    core_of = dst // D
    tile_of = (dst % D) // P
    dloc = dst % P  # D % P == 0, so dst-local-in-tile == dst % P
    win_of = src // WS
    widx = (src % WS).astype(np.int16)

    nkeys = NCORES * T * NWIN
    key = (core_of * T + tile_of) * NWIN + win_of
    counts = np.bincount(key, minlength=nkeys).reshape(NCORES, T, NWIN)

    # chunks per window, shared by every (core, tile): the static schedule
    K_w = np.maximum(1, -(-counts.max(axis=(0, 1)) // P)).astype(np.int64)
    CK = int(K_w.sum())
    woff = np.concatenate([[0], np.cumsum(K_w)[:-1]]).astype(np.int64)

    groups = [G_TILES] * (T // G_TILES)
    if T % G_TILES:
        groups.append(T % G_TILES)
    gstart = np.concatenate([[0], np.cumsum(groups)[:-1]]).astype(np.int64)

    # idx tensor column base per (group, window); cols are int16 columns of a
    # [128, TOTCOL] tensor, 16 indices per column (wrapped-16 layout)
    colbase = np.zeros((len(groups), NWIN), np.int64)
    acc = 0
    for g, Gg in enumerate(groups):
        for w in range(NWIN):
            colbase[g, w] = acc
            acc += Gg * int(K_w[w]) * (P // 16)
    TOTCOL = acc

    # rank of each edge within its (core,tile,window) bucket
    # (src-sorted bucket order was tried and is timing-neutral: the gather
    # is SDMA descriptor-rate bound, not HBM-locality bound)
    order = np.argsort(key, kind="stable")
    starts = np.concatenate([[0], np.cumsum(counts.reshape(-1))])[:-1]
    rank = np.arange(E, dtype=np.int64) - starts[key[order]]

    c_s = core_of[order]
    t_s = tile_of[order]
    w_s = win_of[order]
    k_s = rank // P
    p_s = rank % P

    # gather index tensor (int16, wrapped in 16 rows, replicated x8 later)
    idx16 = np.zeros((NCORES, 16, TOTCOL), np.int16)
    g_s = t_s // G_TILES
    tl_s = t_s % G_TILES
    j = (tl_s * K_w[w_s] + k_s) * P + p_s
    col = colbase[g_s, w_s] + j // 16
    idx16[c_s, j % 16, col] = widx[order]
    idx_full = np.ascontiguousarray(np.tile(idx16, (1, 8, 1)))  # [NCORES,128,TOTCOL]

    # dst-local ids per chunk slot ([-1] = padding -> zero one-hot row)
    dstl = np.full((NCORES, P, T * CK), -1.0, np.float32)
    dstl[c_s, p_s, t_s * CK + woff[w_s] + k_s] = dloc[order].astype(np.float32)
    dstl_bf = dstl.astype(ml_dtypes.bfloat16)

    # feature as bf16, padded to NWIN*WS rows
    fh = np.zeros((NWIN * WS, F), ml_dtypes.bfloat16)
    fh[:N] = np.asarray(feature, np.float32).astype(ml_dtypes.bfloat16)

    cfg = dict(
        N=N, T=T, D=D, WS=WS, K_w=[int(x) for x in K_w], CK=CK,
        woff=[int(x) for x in woff], groups=groups,
        gstart=[int(x) for x in gstart], colbase=colbase, TOTCOL=TOTCOL,
    )
    return cfg, fh, idx_full, dstl_bf


def _build(cfg, trivial=False, reps=1, gather_only=False, bulk=False,
           gsplit=1):
    T, CK, TOTCOL, WS = cfg["T"], cfg["CK"], cfg["TOTCOL"], cfg["WS"]
    K_w, woff, groups, gstart = cfg["K_w"], cfg["woff"], cfg["groups"], cfg["gstart"]
    colbase = cfg["colbase"]
    bf16, f32, i16 = mybir.dt.bfloat16, mybir.dt.float32, mybir.dt.int16

    nc = bass.Bass("TRN2", target_bir_lowering=False, debug=False,
                   num_devices=NCORES, num_swdge_queues=NWIN)
    fh_d = nc.dram_tensor("fh", [NWIN * WS, F], bf16, kind="ExternalInput")
    idx_d = nc.dram_tensor("idx", [P, TOTCOL], i16, kind="ExternalInput")
    dstl_d = nc.dram_tensor("dstl", [P, T * CK], bf16, kind="ExternalInput")
    wt_d = nc.dram_tensor("wt", [F, F], bf16, kind="ExternalInput")  # W.T bf16
    b_d = nc.dram_tensor("bias", [F, 1], f32, kind="ExternalInput")
    iota_d = nc.dram_tensor("iota", [P, P], bf16, kind="ExternalInput")
    out_d = nc.dram_tensor("out", [P, T * P], f32, kind="ExternalOutput")

    if trivial:
        # matched-I/O no-op NEFF for test.py's dispatch-floor probe
        with tile.TileContext(nc) as tc:
            with tc.tile_pool(name="tp", bufs=1) as tp:
                t0 = tp.tile([F, F], bf16)
                nc.sync.dma_start(t0[:], wt_d.ap())
                t1 = tp.tile([F, F], f32)
                nc.vector.tensor_copy(t1[:], t0[:])
                nc.sync.dma_start(out_d.ap()[:, 0:F], t1[:])
        return nc

    # dma_gather (InstDMAGatherAnt) lives in the 'mlp' Q7 library; load it
    # before the Tile-scheduled region (same-engine program order holds).
    # This walrus build's visitInstISA needs the pseudo's 64-byte encoding
    # filled in, which plain load_library leaves empty.
    import concourse.bass_isa as bass_isa
    lib_inst = nc.gpsimd.load_library(library_config.mlp)
    _isa = nc.isa
    _po = _isa.get_enum("NEURON_ISA_TPB_PSEUDO_OPCODE")
    _bytes, _fix = bass_isa.isa_struct(
        _isa, _isa.Opcode.NEURON_ISA_TPB_OPCODE_PSEUDO_INST,
        {"pseudo_opcode":
         _po.NEURON_ISA_TPB_PSEUDO_OPCODE_PSEUDO_LIBRARY_RELOAD_INDEX.value,
         "lib_index": library_config.mlp.index})
    assert not _fix
    lib_inst.ins.instr = _bytes

    # One Pool register per distinct gather size (fresh to_reg per call
    # exhausts the register file at 80 calls).
    nidx_regs = {}
    for Gg in set(groups):
        for w in range(NWIN):
            v = Gg * K_w[w] * P // gsplit
            if v not in nidx_regs:
                r = nc.gpsimd.alloc_register(f"nidx_{v}")
                nc.gpsimd.reg_mov(r, v)
                nidx_regs[v] = r

    with tile.TileContext(nc) as tc:
        with (
            tc.tile_pool(name="const", bufs=1) as cpool,
            tc.tile_pool(name="xp", bufs=XBUFS) as xpool,
            tc.tile_pool(name="work", bufs=2) as wpool,
            tc.tile_pool(name="ps", bufs=2, space="PSUM") as ppool,
        ):
            idx_sb = cpool.tile([P, TOTCOL], i16)
            nc.sync.dma_start(idx_sb[:], idx_d.ap())
            dstl_sb = cpool.tile([P, T * CK], bf16)
            nc.sync.dma_start(dstl_sb[:], dstl_d.ap())
            wt_sb = cpool.tile([F, F], bf16)
            nc.sync.dma_start(wt_sb[:], wt_d.ap())
            b_sb = cpool.tile([F, 1], f32)
            nc.sync.dma_start(b_sb[:], b_d.ap())
            iota_sb = cpool.tile([P, P], bf16)
            nc.sync.dma_start(iota_sb[:], iota_d.ap())

            for rep in range(reps):
                for g, Gg in enumerate(groups):
                    gs = gstart[g]
                    xw = []
                    for w in range(NWIN):
                        x = xpool.tile([P, Gg * K_w[w], F], bf16, tag=f"x{w}",
                                       name=f"x{w}_{g}_r{rep}")
                        if bulk:
                            # timing variant: same volume, ideal DMA pattern
                            nidx = Gg * K_w[w] * P
                            nc.sync.dma_start(
                                x[:],
                                fh_d.ap()[w * WS:w * WS + nidx, :]
                                .rearrange("(p k) f -> p k f", p=P),
                            )
                        xw.append(x)
                    if not bulk:
                        # gsplit>1 splits each window's gather into sub-calls
                        # emitted round-robin across the queues, so desc-gen
                        # of the next queue's sub-call proceeds while this
                        # queue's descriptor ring drains
                        for s in range(gsplit):
                            for w in range(NWIN):
                                assert (Gg * K_w[w]) % gsplit == 0, (Gg, K_w[w])
                                nchunk = Gg * K_w[w] // gsplit
                                ns = nchunk * P
                                c0 = int(colbase[g, w])
                                nc.gpsimd.dma_gather(
                                    out_ap=xw[w][
                                        :, s * nchunk:(s + 1) * nchunk, :],
                                    in_ap=fh_d.ap()[w * WS:(w + 1) * WS, :],
                                    idxs_ap=idx_sb[
                                        :, c0 + s * (ns // 16):
                                        c0 + (s + 1) * (ns // 16)],
                                    num_idxs=ns,
                                    num_idxs_reg=nidx_regs[ns],
                                    elem_size=F,
                                    single_packet=False,
                                    queue_num=w,
                                )

                    ot = wpool.tile([P, Gg * P], f32, tag="ot",
                                    name=f"ot_{g}_r{rep}")
                    for tl in range(Gg):
                        t = gs + tl
                        if gather_only:
                            nc.vector.tensor_copy(
                                ot[:, tl * P:(tl + 1) * P], xw[0][:, tl, :])
                            continue
                        s_t = wpool.tile([P, CK * P], bf16, tag="s",
                                         name=f"s_{t}_r{rep}")
                        nc.vector.tensor_tensor(
                            out=s_t[:].rearrange("p (c f) -> p c f", f=P),
                            in0=dstl_sb[:, t * CK:(t + 1) * CK]
                            .rearrange("p (c o) -> p c o", o=1)
                            .to_broadcast([P, CK, P]),
                            in1=iota_sb[:]
                            .rearrange("p (o f) -> p o f", o=1)
                            .to_broadcast([P, CK, P]),
                            op=mybir.AluOpType.is_equal,
                        )
                        # aggT[f, d] += X_chunk[e, f]^T @ S_chunk[e, d]
                        aggT_ps = ppool.tile([F, P], f32, tag="aggT",
                                             name=f"aggT_{t}_r{rep}")
                        for w in range(NWIN):
                            for k in range(K_w[w]):
                                q = woff[w] + k
                                nc.tensor.matmul(
                                    aggT_ps[:],
                                    lhsT=xw[w][:, tl * K_w[w] + k, :],
                                    rhs=s_t[:, q * P:(q + 1) * P],
                                    start=(q == 0),
                                    stop=(q == CK - 1),
                                )
                        aggT_bf = wpool.tile([F, P], bf16, tag="aggTb",
                                             name=f"aggTb_{t}_r{rep}")
                        nc.vector.tensor_copy(aggT_bf[:], aggT_ps[:])
                        ot_p = ppool.tile([P, F], f32, tag="otp",
                                          name=f"otp_{t}_r{rep}")
                        nc.tensor.matmul(ot_p[:], lhsT=wt_sb[:], rhs=aggT_bf[:],
                                         start=True, stop=True)
                        nc.scalar.activation(
                            ot[:, tl * P:(tl + 1) * P], ot_p[:],
                            mybir.ActivationFunctionType.Relu,
                            bias=b_sb[:], scale=1.0,
                        )
                    nc.sync.dma_start(out_d.ap()[:, gs * P:(gs + Gg) * P], ot[:])
    return nc


def _in_maps(fh, idx_full, dstl_bf, W, b):
    wt = np.ascontiguousarray(np.asarray(W, np.float32).T
                              .astype(ml_dtypes.bfloat16))
    bcol = np.ascontiguousarray(np.asarray(b, np.float32).reshape(F, 1))
    iota = np.ascontiguousarray(
        np.tile(np.arange(P, dtype=np.float32)[None, :], (P, 1))
    ).astype(ml_dtypes.bfloat16)
    return [{
        "fh": fh,
        "idx": np.ascontiguousarray(idx_full[c]),
        "dstl": np.ascontiguousarray(dstl_bf[c]),
        "wt": wt,
        "bias": bcol,
        "iota": iota,
    } for c in range(NCORES)]


def kernel(feature, src, dst, W, b):
    global LAST_NC, LAST_IN_MAPS, LAST_CFG
    feature = np.asarray(feature)
    src = np.asarray(src)
    dst = np.asarray(dst)

    cfg, fh, idx_full, dstl_bf = _prep(feature, src, dst)
    in_maps = _in_maps(fh, idx_full, dstl_bf, W, b)

    # the walrus build in this container sporadically fails codegen; a fresh
    # build + recompile usually succeeds, so retry once before giving up
    results = None
    last_exc = None
    for attempt in range(2):
        try:
            nc = _build(cfg)
            _split_excess_waits(nc)
            LAST_NC, LAST_IN_MAPS, LAST_CFG = nc, in_maps, cfg
            res = run_bass_kernel_spmd(nc, in_maps, core_ids=list(range(NCORES)))
            results = res.results
            break
        except Exception as e:
            last_exc = e
    if results is None:
        raise last_exc

    D = cfg["D"]
    N = cfg["N"]
    out = np.empty((NCORES * D, F), np.float32)
    for c in range(NCORES):
        out[c * D:(c + 1) * D] = results[c]["out"].T
    return np.ascontiguousarray(out[:N])


# revision 17
# speedup vs baseline: 1.1363x; 1.0154x over previous
"""Trainium2 Bass kernel for a GCN layer:

    out = relu(segment_sum(feature[src], dst, N) @ W.T + b)

Strategy (8 NeuronCores, SPMD, no collectives):
  - Destination nodes are sharded across the 8 cores (12544 rows/core in
    tiles of 128). Each core owns all edges whose dst falls in its range.
  - Host prep buckets each core's edges by (dst tile, src window) and pads
    each bucket to a whole number of 128-edge chunks, giving every core an
    identical static schedule (single SPMD NEFF).
  - On device, each group of 5 dst tiles gathers its source rows from HBM
    with one dma_gather per src window (features stored bf16, 256B rows).
  - Per dst tile, a one-hot matmul segment-sums each 128-edge chunk
    directly in transposed orientation:
        aggT[f, d] += X_chunk[128e, f]^T @ S_chunk[128e, d]   (PSUM fp32)
    where S is built on the fly by comparing dst-local ids against an iota.
    This orientation makes the W matmul consume aggT with no transpose:
        out_tile[o, d] = relu(W[o,f] @ aggT[f, d] + b[o])     (bf16 PE)
  - Output is produced transposed per core ([128, 12544]) and re-assembled
    on the host. End-to-end rel err vs fp32 reference ~2.9e-3.
"""

import math

import ml_dtypes
import numpy as np

import concourse.bass as bass
import concourse.mybir as mybir
import concourse.tile as tile
from concourse import library_config
from concourse.bass_utils import run_bass_kernel_spmd

P = 128
F = 128
NCORES = 8
NWIN = 4  # src windows (dma_gather indices are int16, so <=32768 rows each)
G_TILES = 5  # dst tiles processed per group (bounds SBUF working set)
XBUFS = 3  # gather tile double/triple buffering depth

LAST_NC = None  # hooks for test.py's timing harness
LAST_IN_MAPS = None
LAST_CFG = None

SYNC_BUDGET = 1  # this walrus build rejects extra sync commands per inst


def _split_excess_waits(nc, budget=SYNC_BUDGET):
    """Walrus codegen here rejects instructions carrying more than `budget`
    total sync commands (sem waits + updates). Hoist excess waits onto NOPs
    inserted just before the instruction on the same engine (sequencers
    execute in order, so this is semantically identical)."""
    nsplit = 0
    for fn in nc.m.functions:
        for bb in fn.blocks:
            out = []
            for inst in bb.instructions:
                si = inst.sync_info
                if si is None or not si.on_wait:
                    out.append(inst)
                    continue
                allowed = max(0, budget - len(si.on_update))
                if len(si.on_wait) > allowed:
                    waits = list(si.on_wait)
                    excess = waits[allowed:]
                    del si.on_wait[allowed:]
                    for i in range(0, len(excess), budget):
                        n = mybir.InstNoOp(
                            name=f"{inst.name}-waitsplit-{i}", ins=[], outs=[])
                        n.engine = inst.engine
                        n.sync_info = mybir.SyncInfo(
                            on_wait=list(excess[i:i + budget]), on_update=[])
                        out.append(n)
                        nsplit += 1
                out.append(inst)
            bb.instructions[:] = out
    return nsplit


def _prep(feature, src, dst):
    """Bucket edges per (core, dst tile, src window); build per-core gather
    index / dst-local tensors with a schedule shared by all cores."""
    N = feature.shape[0]
    E = src.shape[0]
    T = math.ceil(N / (NCORES * P))  # dst tiles per core
    D = T * P  # dst rows per core
    WS = math.ceil(N / NWIN)  # src window rows
    assert WS <= 32768, f"window {WS} exceeds int16 gather index range"

    src = np.asarray(src, np.int64)
    dst = np.asarray(dst, np.int64)

    core_of = dst // D
    tile_of = (dst % D) // P
    dloc = dst % P  # D % P == 0, so dst-local-in-tile == dst % P
    win_of = src // WS
    widx = (src % WS).astype(np.int16)

    nkeys = NCORES * T * NWIN
    key = (core_of * T + tile_of) * NWIN + win_of
    counts = np.bincount(key, minlength=nkeys).reshape(NCORES, T, NWIN)

    # chunks per window, shared by every (core, tile): the static schedule
    K_w = np.maximum(1, -(-counts.max(axis=(0, 1)) // P)).astype(np.int64)
    CK = int(K_w.sum())
    woff = np.concatenate([[0], np.cumsum(K_w)[:-1]]).astype(np.int64)

    groups = [G_TILES] * (T // G_TILES)
    if T % G_TILES:
        groups.append(T % G_TILES)
    gstart = np.concatenate([[0], np.cumsum(groups)[:-1]]).astype(np.int64)

    # idx tensor column base per (group, window); cols are int16 columns of a
    # [128, TOTCOL] tensor, 16 indices per column (wrapped-16 layout)
    colbase = np.zeros((len(groups), NWIN), np.int64)
    acc = 0
    for g, Gg in enumerate(groups):
        for w in range(NWIN):
            colbase[g, w] = acc
            acc += Gg * int(K_w[w]) * (P // 16)
    TOTCOL = acc

    # rank of each edge within its (core,tile,window) bucket
    # (src-sorted bucket order was tried and is timing-neutral: the gather
    # is SDMA descriptor-rate bound, not HBM-locality bound)
    order = np.argsort(key, kind="stable")
    starts = np.concatenate([[0], np.cumsum(counts.reshape(-1))])[:-1]
    rank = np.arange(E, dtype=np.int64) - starts[key[order]]

    c_s = core_of[order]
    t_s = tile_of[order]
    w_s = win_of[order]
    k_s = rank // P
    p_s = rank % P

    # gather index tensor (int16, wrapped in 16 rows, replicated x8 later)
    idx16 = np.zeros((NCORES, 16, TOTCOL), np.int16)
    g_s = t_s // G_TILES
    tl_s = t_s % G_TILES
    j = (tl_s * K_w[w_s] + k_s) * P + p_s
    col = colbase[g_s, w_s] + j // 16
    idx16[c_s, j % 16, col] = widx[order]
    idx_full = np.ascontiguousarray(np.tile(idx16, (1, 8, 1)))  # [NCORES,128,TOTCOL]

    # dst-local ids per chunk slot ([-1] = padding -> zero one-hot row)
    dstl = np.full((NCORES, P, T * CK), -1.0, np.float32)
    dstl[c_s, p_s, t_s * CK + woff[w_s] + k_s] = dloc[order].astype(np.float32)
    dstl_bf = dstl.astype(ml_dtypes.bfloat16)

    # feature as bf16, padded to NWIN*WS rows
    fh = np.zeros((NWIN * WS, F), ml_dtypes.bfloat16)
    fh[:N] = np.asarray(feature, np.float32).astype(ml_dtypes.bfloat16)

    cfg = dict(
        N=N, T=T, D=D, WS=WS, K_w=[int(x) for x in K_w], CK=CK,
        woff=[int(x) for x in woff], groups=groups,
        gstart=[int(x) for x in gstart], colbase=colbase, TOTCOL=TOTCOL,
    )
    return cfg, fh, idx_full, dstl_bf


def _build(cfg, trivial=False, reps=1, gather_only=False, bulk=False,
           gsplit=1, single_packet=False):
    T, CK, TOTCOL, WS = cfg["T"], cfg["CK"], cfg["TOTCOL"], cfg["WS"]
    K_w, woff, groups, gstart = cfg["K_w"], cfg["woff"], cfg["groups"], cfg["gstart"]
    colbase = cfg["colbase"]
    bf16, f32, i16 = mybir.dt.bfloat16, mybir.dt.float32, mybir.dt.int16

    nc = bass.Bass("TRN2", target_bir_lowering=False, debug=False,
                   num_devices=NCORES, num_swdge_queues=NWIN)
    fh_d = nc.dram_tensor("fh", [NWIN * WS, F], bf16, kind="ExternalInput")
    idx_d = nc.dram_tensor("idx", [P, TOTCOL], i16, kind="ExternalInput")
    dstl_d = nc.dram_tensor("dstl", [P, T * CK], bf16, kind="ExternalInput")
    wt_d = nc.dram_tensor("wt", [F, F], bf16, kind="ExternalInput")  # W.T bf16
    b_d = nc.dram_tensor("bias", [F, 1], f32, kind="ExternalInput")
    iota_d = nc.dram_tensor("iota", [P, P], bf16, kind="ExternalInput")
    out_d = nc.dram_tensor("out", [P, T * P], f32, kind="ExternalOutput")

    if trivial:
        # matched-I/O no-op NEFF for test.py's dispatch-floor probe
        with tile.TileContext(nc) as tc:
            with tc.tile_pool(name="tp", bufs=1) as tp:
                t0 = tp.tile([F, F], bf16)
                nc.sync.dma_start(t0[:], wt_d.ap())
                t1 = tp.tile([F, F], f32)
                nc.vector.tensor_copy(t1[:], t0[:])
                nc.sync.dma_start(out_d.ap()[:, 0:F], t1[:])
        return nc

    # dma_gather (InstDMAGatherAnt) lives in the 'mlp' Q7 library; load it
    # before the Tile-scheduled region (same-engine program order holds).
    # This walrus build's visitInstISA needs the pseudo's 64-byte encoding
    # filled in, which plain load_library leaves empty.
    import concourse.bass_isa as bass_isa
    lib_inst = nc.gpsimd.load_library(library_config.mlp)
    _isa = nc.isa
    _po = _isa.get_enum("NEURON_ISA_TPB_PSEUDO_OPCODE")
    _bytes, _fix = bass_isa.isa_struct(
        _isa, _isa.Opcode.NEURON_ISA_TPB_OPCODE_PSEUDO_INST,
        {"pseudo_opcode":
         _po.NEURON_ISA_TPB_PSEUDO_OPCODE_PSEUDO_LIBRARY_RELOAD_INDEX.value,
         "lib_index": library_config.mlp.index})
    assert not _fix
    lib_inst.ins.instr = _bytes

    # One Pool register per distinct gather size (fresh to_reg per call
    # exhausts the register file at 80 calls).
    nidx_regs = {}
    for Gg in set(groups):
        for w in range(NWIN):
            Gk = Gg * K_w[w]
            for s in range(gsplit):
                v = ((s + 1) * Gk // gsplit - s * Gk // gsplit) * P
                if v not in nidx_regs:
                    r = nc.gpsimd.alloc_register(f"nidx_{v}")
                    nc.gpsimd.reg_mov(r, v)
                    nidx_regs[v] = r

    with tile.TileContext(nc) as tc:
        with (
            tc.tile_pool(name="const", bufs=1) as cpool,
            tc.tile_pool(name="xp", bufs=XBUFS) as xpool,
            tc.tile_pool(name="work", bufs=2) as wpool,
            tc.tile_pool(name="ps", bufs=2, space="PSUM") as ppool,
        ):
            idx_sb = cpool.tile([P, TOTCOL], i16)
            nc.sync.dma_start(idx_sb[:], idx_d.ap())
            dstl_sb = cpool.tile([P, T * CK], bf16)
            nc.sync.dma_start(dstl_sb[:], dstl_d.ap())
            wt_sb = cpool.tile([F, F], bf16)
            nc.sync.dma_start(wt_sb[:], wt_d.ap())
            b_sb = cpool.tile([F, 1], f32)
            nc.sync.dma_start(b_sb[:], b_d.ap())
            iota_sb = cpool.tile([P, P], bf16)
            nc.sync.dma_start(iota_sb[:], iota_d.ap())

            for rep in range(reps):
                for g, Gg in enumerate(groups):
                    gs = gstart[g]
                    xw = []
                    for w in range(NWIN):
                        x = xpool.tile([P, Gg * K_w[w], F], bf16, tag=f"x{w}",
                                       name=f"x{w}_{g}_r{rep}")
                        if bulk:
                            # timing variant: same volume, ideal DMA pattern
                            nidx = Gg * K_w[w] * P
                            nc.sync.dma_start(
                                x[:],
                                fh_d.ap()[w * WS:w * WS + nidx, :]
                                .rearrange("(p k) f -> p k f", p=P),
                            )
                        xw.append(x)
                    if not bulk:
                        # gsplit>1 splits each window's gather into sub-calls
                        # (uneven integer splits allowed), emitted round-robin
                        # across the queues. With single_packet=True each
                        # engine's descriptor stream coalesces into one packet
                        # (amortizes per-packet SDMA overhead; needs sub-calls
                        # under the ~2K-index fault threshold).
                        for s in range(gsplit):
                            for w in range(NWIN):
                                Gk = Gg * K_w[w]
                                lo = s * Gk // gsplit
                                hi = (s + 1) * Gk // gsplit
                                ns = (hi - lo) * P
                                c0 = int(colbase[g, w])
                                nc.gpsimd.dma_gather(
                                    out_ap=xw[w][:, lo:hi, :],
                                    in_ap=fh_d.ap()[w * WS:(w + 1) * WS, :],
                                    idxs_ap=idx_sb[
                                        :, c0 + lo * (P // 16):
                                        c0 + hi * (P // 16)],
                                    num_idxs=ns,
                                    num_idxs_reg=nidx_regs[ns],
                                    elem_size=F,
                                    single_packet=single_packet,
                                    queue_num=w,
                                )

                    ot = wpool.tile([P, Gg * P], f32, tag="ot",
                                    name=f"ot_{g}_r{rep}")
                    for tl in range(Gg):
                        t = gs + tl
                        if gather_only:
                            nc.vector.tensor_copy(
                                ot[:, tl * P:(tl + 1) * P], xw[0][:, tl, :])
                            continue
                        s_t = wpool.tile([P, CK * P], bf16, tag="s",
                                         name=f"s_{t}_r{rep}")
                        nc.vector.tensor_tensor(
                            out=s_t[:].rearrange("p (c f) -> p c f", f=P),
                            in0=dstl_sb[:, t * CK:(t + 1) * CK]
                            .rearrange("p (c o) -> p c o", o=1)
                            .to_broadcast([P, CK, P]),
                            in1=iota_sb[:]
                            .rearrange("p (o f) -> p o f", o=1)
                            .to_broadcast([P, CK, P]),
                            op=mybir.AluOpType.is_equal,
                        )
                        # aggT[f, d] += X_chunk[e, f]^T @ S_chunk[e, d]
                        aggT_ps = ppool.tile([F, P], f32, tag="aggT",
                                             name=f"aggT_{t}_r{rep}")
                        for w in range(NWIN):
                            for k in range(K_w[w]):
                                q = woff[w] + k
                                nc.tensor.matmul(
                                    aggT_ps[:],
                                    lhsT=xw[w][:, tl * K_w[w] + k, :],
                                    rhs=s_t[:, q * P:(q + 1) * P],
                                    start=(q == 0),
                                    stop=(q == CK - 1),
                                )
                        aggT_bf = wpool.tile([F, P], bf16, tag="aggTb",
                                             name=f"aggTb_{t}_r{rep}")
                        nc.vector.tensor_copy(aggT_bf[:], aggT_ps[:])
                        ot_p = ppool.tile([P, F], f32, tag="otp",
                                          name=f"otp_{t}_r{rep}")
                        nc.tensor.matmul(ot_p[:], lhsT=wt_sb[:], rhs=aggT_bf[:],
                                         start=True, stop=True)
                        nc.scalar.activation(
                            ot[:, tl * P:(tl + 1) * P], ot_p[:],
                            mybir.ActivationFunctionType.Relu,
                            bias=b_sb[:], scale=1.0,
                        )
                    nc.sync.dma_start(out_d.ap()[:, gs * P:(gs + Gg) * P], ot[:])
    return nc


def _in_maps(fh, idx_full, dstl_bf, W, b):
    wt = np.ascontiguousarray(np.asarray(W, np.float32).T
                              .astype(ml_dtypes.bfloat16))
    bcol = np.ascontiguousarray(np.asarray(b, np.float32).reshape(F, 1))
    iota = np.ascontiguousarray(
        np.tile(np.arange(P, dtype=np.float32)[None, :], (P, 1))
    ).astype(ml_dtypes.bfloat16)
    return [{
        "fh": fh,
        "idx": np.ascontiguousarray(idx_full[c]),
        "dstl": np.ascontiguousarray(dstl_bf[c]),
        "wt": wt,
        "bias": bcol,
        "iota": iota,
    } for c in range(NCORES)]


def kernel(feature, src, dst, W, b):
    global LAST_NC, LAST_IN_MAPS, LAST_CFG
    feature = np.asarray(feature)
    src = np.asarray(src)
    dst = np.asarray(dst)

    cfg, fh, idx_full, dstl_bf = _prep(feature, src, dst)
    in_maps = _in_maps(fh, idx_full, dstl_bf, W, b)

    # the walrus build in this container sporadically fails codegen; a fresh
    # build + recompile usually succeeds, so retry once before giving up
    results = None
    last_exc = None
    for attempt in range(2):
        try:
            nc = _build(cfg)
            _split_excess_waits(nc)
            LAST_NC, LAST_IN_MAPS, LAST_CFG = nc, in_maps, cfg
            res = run_bass_kernel_spmd(nc, in_maps, core_ids=list(range(NCORES)))
            results = res.results
            break
        except Exception as e:
            last_exc = e
    if results is None:
        raise last_exc

    D = cfg["D"]
    N = cfg["N"]
    out = np.empty((NCORES * D, F), np.float32)
    for c in range(NCORES):
        out[c * D:(c + 1) * D] = results[c]["out"].T
    return np.ascontiguousarray(out[:N])
